# revision 2
# baseline (speedup 1.0000x reference)
"""Multi-head attention (B=4, T=2048, H=1024, nh=16) on 8 Trainium2 cores.

Sharding: core = (batch b, head-group g); 4 batches x 2 groups of 8 heads.
Each core computes Q^T/K^T projections for its 512 head-dims, the V
projection (shipped to HBM), and per head the softmax-weighted column
sums cbar[s] = sum_t exp(scores[t,s])/denom[t].  Because the reference
takes mean over T before the output projection, the full [T,T]x[T,dh]
context matmul collapses: ctx_mean[d] = (1/T) sum_s cbar[s] V[s,d],
which the host finishes along with the (tiny) Wo projection.

vs the original baseline:
- ONE merged bf16 input [1024, 3584] (xT | WqT | WkT | WvT) and ONE
  merged bf16 output [520, 2048] (vout rows 0:512, cbar rows 512:520):
  halves DMA bytes and cuts per-call dispatch overhead
- exp without accum_out (saves the ~187ns/instr accumulator-read on the
  ACT critical path); denominators via 2x-rate bf16 tree-folds on DVE
- cbar accumulators at partitions {0,32} of two banks -> the M=1
  matmuls run as 2 concurrent col-tiles (positions {64,96} measured
  slower - quadrant-3 issue)
- projection/V work split into <=4-matmul chunks interleaved between
  scores so the ACT exp stream never queues behind a long PE chain
"""

import numpy as np

B, T, C = 4, 2048, 1024
NH, DH = 16, 64
HLOC = 8          # heads per core
D = HLOC * DH     # 512 projection dims per core
N_CORES = 8

C_TILES = C // 128    # 8
T_TILES = T // 128    # 16
IN_W = T + 3 * D      # 3584 merged input columns per c-row
VROWS = T * D // 2048  # 512 rows of vout in the merged output

_CACHE = {}


def _build(do_cbar=True, do_exp=True):
    import concourse.mybir as mybir
    import concourse.tile as tile
    from concourse import bacc

    f32 = mybir.dt.float32
    bf16 = mybir.dt.bfloat16
    Exp = mybir.ActivationFunctionType.Exp
    AxX = mybir.AxisListType.X
    Add = mybir.AluOpType.add

    nc = bacc.Bacc("TRN2", target_bir_lowering=False, debug=False,
                   num_devices=N_CORES)

    IN = nc.dram_tensor("inp", [C, IN_W], bf16, kind="ExternalInput").ap()
    OUT = nc.dram_tensor("out", [VROWS + HLOC, 2048], bf16,
                         kind="ExternalOutput").ap()

    def in_x(c, lo, hi):
        return IN[c * 128:(c + 1) * 128, lo:hi]

    def in_w(c, which):     # 0=q 1=k 2=v
        base = T + which * D
        return IN[c * 128:(c + 1) * 128, base:base + D]

    with tile.TileContext(nc) as tc, \
         nc.allow_low_precision("bf16 attention tolerated by 2e-2 rel-err"):
        with tc.tile_pool(name="load", bufs=1) as load, \
             tc.tile_pool(name="qtkt", bufs=2) as qtkt, \
             tc.tile_pool(name="wpool", bufs=6) as wpool, \
             tc.tile_pool(name="small", bufs=4) as small, \
             tc.tile_pool(name="stage", bufs=1) as stage_pool, \
             tc.tile_pool(name="vstage", bufs=2) as vstage:

            xt_all = load.tile([128, C_TILES * T], bf16)
            wq_all = load.tile([128, C_TILES * D], bf16)
            wk_all = load.tile([128, C_TILES * D], bf16)
            wv_all = load.tile([128, C_TILES * D], bf16)
            cstageA = stage_pool.tile([128, HLOC * 512], bf16, name="cstA")
            cstageB = stage_pool.tile([128, HLOC * 512], bf16, name="cstB")
            # persistent cbar stage: two areas (one per accumulator bank),
            # head H -> cols [H*512,(H+1)*512), rows {0,32} = the 2 s-blocks
            # of that bank (partition-aligned with the psum accumulators)

            # touch the exp table set first so its ~2.7us load runs during
            # the input DMA
            preheat = small.tile([128, 1], f32, tag="preheat")
            nc.gpsimd.memset(preheat[:], 0.0)
            nc.scalar.activation(preheat[:], preheat[:], Exp)
            # PE warm-up fodder: zeros, so warm matmuls can ride inside a
            # live accumulation group as +0 contributions
            warm = small.tile([128, 640], bf16, tag="warm")
            nc.gpsimd.memset(warm[:], 0.0)
            # DMA order: (wk_c, wq_c, xt_c) per c so the pair-0 projection
            # chains can start as soon as their c-tiles land; wv last (V
            # projection runs late).  sync + scalar queues (ACT is idle
            # during the load phase).
            for c in range(C_TILES):
                eng_w = nc.sync if c % 2 == 0 else nc.scalar
                eng_x = nc.scalar if c % 2 == 0 else nc.sync
                eng_w.dma_start(wk_all[:, c * D:(c + 1) * D], in_w(c, 1))
                eng_w.dma_start(wq_all[:, c * D:(c + 1) * D], in_w(c, 0))
                eng_x.dma_start(xt_all[:, c * T: c * T + 1024],
                                in_x(c, 0, 1024))
                eng_w.dma_start(xt_all[:, c * T + 1024: (c + 1) * T],
                                in_x(c, 1024, 2048))
            for c in range(C_TILES):
                eng = nc.sync if c % 2 == 0 else nc.scalar
                eng.dma_start(wv_all[:, c * D:(c + 1) * D], in_w(c, 2))

            # per-pair Q^T/K^T tiles [128, T]; rows = 2 heads x 64 dims
            qt, kt = {}, {}

            def alloc_pair(p):
                qt[p] = qtkt.tile([128, T], bf16, tag="qt", name=f"qt{p}")
                kt[p] = qtkt.tile([128, T], bf16, tag="kt", name=f"kt{p}")

            def proj_mm(psum_ap, w_all, dt_, tb, c):
                nc.tensor.matmul(
                    psum_ap,
                    w_all[:, c * D + dt_ * 128: c * D + (dt_ + 1) * 128],
                    xt_all[:, c * T + tb * 512: c * T + tb * 512 + 512],
                    start=(c == 0), stop=(c == C_TILES - 1))

            # ---- prefix: Q^T/K^T for pair 0 ----
            alloc_pair(0)
            with tc.tile_pool(name="proj_ps", bufs=1, space="PSUM") as proj_ps:
                pq = [proj_ps.tile([128, 512], f32, tag=f"ppq{i}", bufs=1,
                                   name=f"ppq{i}") for i in range(4)]
                pk = [proj_ps.tile([128, 512], f32, tag=f"ppk{i}", bufs=1,
                                   name=f"ppk{i}") for i in range(4)]
                # dummy matmuls on the warm tile fill the PE's DMA-wait
                # gaps during the load phase so the HAM clock-gate ramps to
                # 2.4GHz and stays there before the attention stream starts
                def warm_mm(n, start):
                    for i in range(n):
                        nc.tensor.matmul(pq[0][:], warm[:, 0:128],
                                         warm[:, 128:640],
                                         start=(start and i == 0),
                                         stop=False, skip_group_check=True)

                warm_mm(4, True)   # opens the pq0 group with +0 terms
                for c in range(C_TILES):
                    for tb in range(4):
                        proj_mm(pk[tb][:], wk_all, 0, tb, c)
                    for tb in range(4):
                        if tb == 0:
                            # pq0's real chain joins the already-open group
                            nc.tensor.matmul(
                                pq[0][:],
                                wq_all[:, c * D: c * D + 128],
                                xt_all[:, c * T: c * T + 512],
                                start=False, stop=(c == C_TILES - 1),
                                skip_group_check=True)
                        else:
                            proj_mm(pq[tb][:], wq_all, 0, tb, c)
                    if c < C_TILES - 1:
                        warm_mm(3, False)
                for tb in range(4):
                    nc.vector.tensor_copy(kt[0][:, tb * 512:(tb + 1) * 512],
                                          pk[tb][:])
                    nc.vector.tensor_copy(qt[0][:, tb * 512:(tb + 1) * 512],
                                          pq[tb][:])
                alloc_pair(1)

            # ---- attention ----
            with tc.tile_pool(name="score_ps", bufs=2, space="PSUM") as score_ps, \
                 tc.tile_pool(name="cb_ps", bufs=2, space="PSUM") as cb_ps, \
                 tc.tile_pool(name="pj_ps", bufs=2, space="PSUM") as pj_ps:

                def mk_qtkt_chunks(w_all_n, p_n, tb):
                    """q or k projection group (dt=p_n, t-block tb) as two
                    4-matmul chunks sharing one psum tile + final copy."""
                    state = {}

                    def chunk1():
                        state["pj"] = pj_ps.tile([128, 512], f32, tag="pj",
                                                 name=f"pj{p_n}{tb}")
                        for c in range(4):
                            proj_mm(state["pj"][:], w_all_n, p_n, tb, c)

                    def chunk2():
                        dst = (qt if w_all_n is wq_all else kt)[p_n]
                        for c in range(4, C_TILES):
                            proj_mm(state["pj"][:], w_all_n, p_n, tb, c)
                        nc.vector.tensor_copy(
                            dst[:, tb * 512:(tb + 1) * 512], state["pj"][:])

                    return [chunk1, chunk2]

                def mk_v_chunks(tt_v):
                    """V projection for t-block tt_v as two 4-matmul chunks
                    + copy + dma into the merged output rows."""
                    state = {}

                    def vmm(c):
                        nc.tensor.matmul(
                            state["pj"][:],
                            xt_all[:, c * T + tt_v * 128: c * T + (tt_v + 1) * 128],
                            wv_all[:, c * D:(c + 1) * D],
                            start=(c == 0), stop=(c == C_TILES - 1))

                    def chunk1():
                        state["pj"] = pj_ps.tile([128, 512], f32, tag="pj",
                                                 name=f"pv{tt_v}")
                        for c in range(4):
                            vmm(c)

                    def chunk2():
                        for c in range(4, C_TILES):
                            vmm(c)
                        vs = vstage.tile([128, D], bf16)
                        nc.vector.tensor_copy(vs[:], state["pj"][:])
                        nc.sync.dma_start(
                            OUT[tt_v * 32:(tt_v + 1) * 32, :], vs[:])

                    return [chunk1, chunk2]

                boundary_sched = {}
                boundary_sched[0] = sum(
                    [mk_qtkt_chunks(wq_all, 1, tb) for tb in range(4)], [])
                boundary_sched[1] = sum(
                    [mk_qtkt_chunks(wk_all, 1, tb) for tb in range(4)], []) + \
                    sum([mk_v_chunks(v) for v in (0, 1)], [])
                boundary_sched[2] = sum(
                    [mk_qtkt_chunks(wq_all, 2, tb) for tb in range(4)], []) + \
                    sum([mk_v_chunks(v) for v in (2, 3)], [])
                boundary_sched[3] = sum(
                    [mk_qtkt_chunks(wk_all, 2, tb) for tb in range(4)], []) + \
                    sum([mk_v_chunks(v) for v in (4, 5)], [])
                boundary_sched[4] = sum(
                    [mk_qtkt_chunks(wq_all, 3, tb) for tb in range(4)], []) + \
                    sum([mk_v_chunks(v) for v in (6, 7)], [])
                boundary_sched[5] = sum(
                    [mk_qtkt_chunks(wk_all, 3, tb) for tb in range(4)], []) + \
                    sum([mk_v_chunks(v) for v in (8, 9)], [])
                boundary_sched[6] = sum(
                    [mk_v_chunks(v) for v in (10, 11, 12)], [])
                boundary_sched[7] = sum(
                    [mk_v_chunks(v) for v in (13, 14, 15)], [])

                for H in range(HLOC):
                    pair = H // 2
                    odd = H % 2
                    row0 = 64 * odd
                    if pair in (1, 2) and odd == 0:
                        alloc_pair(pair + 1)
                    chunks = boundary_sched[H]
                    # cbar accumulators: j={0,1} in bank A rows {0,32},
                    # j={2,3} in bank B rows {0,32} -> pairs of concurrent
                    # col-tiled matmuls
                    cba = cb_ps.tile([128, 512], f32, tag="cb", name=f"cba{H}")
                    cbb = cb_ps.tile([128, 512], f32, tag="cb", name=f"cbb{H}")
                    cb_slot = [(cba, 0), (cba, 32), (cbb, 0), (cbb, 32)]
                    pending = []

                    def emit_cbar(p_tt, p_r, p_w):
                        for j in range(4):
                            bank, row = cb_slot[j]
                            nc.tensor.matmul(
                                bank[row:row + 1, :], p_r[:],
                                p_w[:, j * 512:(j + 1) * 512],
                                start=(p_tt == 0), stop=(p_tt == T_TILES - 1),
                                tile_position=(0, row))

                    for tt in range(T_TILES):
                        qs = qt[pair][row0:row0 + 64, tt * 128:(tt + 1) * 128]
                        sc = [score_ps.tile([128, 1024], f32, tag="sc",
                                            name=f"sc{i}") for i in range(2)]
                        for i in range(2):
                            for j in range(2):
                                s_blk = i * 2 + j
                                nc.tensor.matmul(
                                    sc[i][:, j * 512:(j + 1) * 512],
                                    qs,
                                    kt[pair][row0:row0 + 64,
                                             s_blk * 512: s_blk * 512 + 512],
                                    start=True, stop=True)
                        if tt < len(chunks):
                            chunks[tt]()
                        # deferred cbar so the PE stream never blocks
                        # waiting on this unit's r (tighter on the last
                        # head to shorten the tail flush)
                        lag = 1 if (H == HLOC - 1 and tt >= 12) else 2
                        while do_cbar and len(pending) > lag:
                            emit_cbar(*pending.pop(0))

                        if not do_exp:
                            continue
                        last_unit = (H == HLOC - 1 and tt == T_TILES - 1)
                        w = wpool.tile([128, T], bf16)
                        denom = small.tile([128, 1], f32, tag="denom")
                        if last_unit:
                            # accum_out path: shortest possible tail chain
                            accs = small.tile([128, 2], f32, tag="accs")
                            for i in range(2):
                                nc.scalar.activation(
                                    w[:, i * 1024:(i + 1) * 1024], sc[i][:],
                                    Exp, scale=0.125,
                                    accum_out=accs[:, i:i + 1])
                            nc.vector.tensor_add(denom[:], accs[:, 0:1],
                                                 accs[:, 1:2])
                        else:
                            for i in range(2):
                                nc.scalar.activation(
                                    w[:, i * 1024:(i + 1) * 1024], sc[i][:],
                                    Exp, scale=0.125)
                            # denominators: bf16 tree-folds at 2x DVE rate
                            # (tensor_reduce is 1x), then one short reduce
                            f1 = small.tile([128, 1024], bf16, tag="f1")
                            nc.vector.tensor_add(f1[:], w[:, 0:1024],
                                                 w[:, 1024:2048])
                            f2 = small.tile([128, 512], bf16, tag="f2")
                            nc.vector.tensor_add(f2[:], f1[:, 0:512],
                                                 f1[:, 512:1024])
                            f3 = small.tile([128, 256], bf16, tag="f3")
                            nc.vector.tensor_add(f3[:], f2[:, 0:256],
                                                 f2[:, 256:512])
                            nc.vector.tensor_reduce(denom[:], f3[:], AxX, Add)
                        r32 = small.tile([128, 1], f32, tag="r32")
                        nc.vector.reciprocal(r32[:], denom[:])
                        r = small.tile([128, 1], bf16, tag="r")
                        nc.vector.tensor_copy(r[:], r32[:])
                        pending.append((tt, r, w))
                    if not do_cbar:
                        pending.clear()
                        continue
                    for p in pending:
                        emit_cbar(*p)
                    pending.clear()
                    # evacuate cbar into the persistent stages (partition-
                    # aligned: bank rows {0,32} -> stage rows {0,32}), then
                    # one DMA per bank into this head's cbar output row
                    for j in range(4):
                        bank, row = cb_slot[j]
                        area = cstageA if j < 2 else cstageB
                        nc.vector.tensor_copy(
                            area[row:row + 1, H * 512:(H + 1) * 512],
                            bank[row:row + 1, :])
                    nc.sync.dma_start(
                        OUT[VROWS + H:VROWS + H + 1, 0:1024],
                        cstageA[0:64:32, H * 512:(H + 1) * 512])
                    nc.sync.dma_start(
                        OUT[VROWS + H:VROWS + H + 1, 1024:2048],
                        cstageB[0:64:32, H * 512:(H + 1) * 512])

    nc.compile()
    return nc


def _setup_exec(cache=None, **build_kwargs):
    """Build the Bass module and a cached jitted SPMD executor
    (mirrors concourse.bass2jax.run_bass_via_pjrt's multi-core path)."""
    import jax
    import concourse.mybir as mybir
    from concourse import bass2jax
    from jax.experimental.shard_map import shard_map
    from jax.sharding import Mesh, PartitionSpec

    if cache is None:
        cache = _CACHE
    nc = _build(**build_kwargs)
    bass2jax.install_neuronx_cc_hook()

    partition_name = (nc.partition_id_tensor.name
                      if nc.partition_id_tensor else None)
    in_names, out_names, out_avals, zero_shapes = [], [], [], []
    for alloc in nc.m.functions[0].allocations:
        if not isinstance(alloc, mybir.MemoryLocationSet):
            continue
        name = alloc.memorylocations[0].name
        if alloc.kind == "ExternalInput":
            if name != partition_name:
                in_names.append(name)
        elif alloc.kind == "ExternalOutput":
            shape = tuple(alloc.tensor_shape)
            dtype = mybir.dt.np(alloc.dtype)
            out_names.append(name)
            out_avals.append(jax.core.ShapedArray(shape, dtype))
            zero_shapes.append((shape, dtype))
    n_params = len(in_names)
    all_in_names = in_names + out_names
    if partition_name is not None:
        all_in_names = all_in_names + [partition_name]

    def _body(*args):
        operands = list(args)
        if partition_name is not None:
            operands.append(bass2jax.partition_id_tensor())
        outs = bass2jax._bass_exec_p.bind(
            *operands,
            out_avals=tuple(out_avals),
            in_names=tuple(all_in_names),
            out_names=tuple(out_names),
            lowering_input_output_aliases=(),
            sim_require_finite=True,
            sim_require_nnan=True,
            nc=nc,
        )
        return tuple(outs)

    devices = jax.devices()[:N_CORES]
    mesh = Mesh(np.asarray(devices), ("core",))
    n_outs = len(out_names)
    sharded = jax.jit(
        shard_map(_body, mesh=mesh,
                  in_specs=(PartitionSpec("core"),) * (n_params + n_outs),
                  out_specs=(PartitionSpec("core"),) * n_outs,
                  check_rep=False),
        donate_argnums=tuple(range(n_params, n_params + n_outs)),
        keep_unused=True,
    )

    from jax.sharding import NamedSharding
    shardings = NamedSharding(mesh, PartitionSpec("core"))

    def make_zeros():
        import jax.numpy as jnp
        return [
            jax.device_put(
                jnp.zeros((N_CORES * s[0], *s[1:]), d), shardings)
            for s, d in zero_shapes
        ]

    cache.update(nc=nc, sharded=sharded, in_names=in_names,
                 out_names=out_names, out_avals=out_avals,
                 make_zeros=make_zeros, shardings=shardings)
    return cache


def kernel(x, Wq, Wk, Wv, Wo, bo):
    import jax
    import ml_dtypes

    bfloat16 = ml_dtypes.bfloat16
    x = np.asarray(x, dtype=np.float32)
    Wq = np.asarray(Wq, dtype=np.float32)
    Wk = np.asarray(Wk, dtype=np.float32)
    Wv = np.asarray(Wv, dtype=np.float32)
    Wo = np.asarray(Wo, dtype=np.float32)
    bo = np.asarray(bo, dtype=np.float32)

    if "sharded" not in _CACHE:
        _setup_exec()

    ins = []
    for b in range(B):
        xtb = np.ascontiguousarray(x[b].T)            # [C, T]
        for g in range(2):
            rows = slice(g * D, (g + 1) * D)
            merged = np.concatenate(
                [xtb, Wq[rows, :].T, Wk[rows, :].T, Wv[rows, :].T],
                axis=1).astype(bfloat16)              # [C, 3584]
            ins.append(merged)

    concat_in = [np.concatenate(ins, axis=0)]
    device_inputs = [jax.device_put(a, _CACHE["shardings"]) for a in concat_in]
    _CACHE["device_inputs"] = device_inputs

    out_arrs = _CACHE["sharded"](*device_inputs, *_CACHE["make_zeros"]())
    outmat = np.asarray(out_arrs[0]).reshape(N_CORES, VROWS + HLOC, 2048)

    ctx_mean = np.empty((B, C), dtype=np.float32)
    for core in range(N_CORES):
        b, g = divmod(core, 2)
        om = outmat[core]
        vout = om[:VROWS, :].astype(np.float32).reshape(T, D)
        cbar = om[VROWS:, :].astype(np.float32)       # [8, T]
        v_r = vout.reshape(T, HLOC, DH)
        cm = np.einsum("hs,shd->hd", cbar, v_r, optimize=True) / np.float32(T)
        ctx_mean[b, g * D:(g + 1) * D] = cm.reshape(-1)

    return ctx_mean @ Wo.T + bo
